# revision 20
# baseline (speedup 1.0000x reference)
"""Causal self-attention (B=4, T=2048, C=1024, H=16) on 8 trn2 NeuronCores.

Sharding: head-pair parallel. Core c owns heads {2c, 2c+1} for all 4 batches.
 - host: x is pre-transposed to xT [C, B*T]; W_qkv is pre-sliced per core into
   wq/wk/wv [C, 128] (2 heads x 64, softmax scale folded into wq), W_proj and
   biases broadcast.
 - device per core: qkv projections as fp32r matmuls producing qT/kT [d2, T]
   (d on partitions) and vT [d2, T]; vT is PE-transposed per 128-tile into
   v [T, 64]-per-head tiles with an appended ones column.
 - attention in S^T orientation: S^T[tk, tq] = kT.T@qT tiles [128, 512] with the
   causal mask preloaded into PSUM via an identity matmul; softmax without
   max-subtraction (|S| <= ~20, safe in fp32): P^T = exp(S^T) on ScalarE
   (PSUM->SBUF, rounded to f32r). O-matmul lhsT = [v_h | ones] (M=65) yields
   both O^T[d, tq] and the denominator row l in one pass. Normalize via
   reciprocal + K=1 broadcast matmul + DVE multiply.
 - per-batch AllToAll (1 MB/rank) reshards O^T from head-shards to
   token-shards; column-parallel out-projection with fused bias produces
   out^T [C, 1024 tokens] per core; host reassembles.
"""
import numpy as np
import concourse.bacc as bacc
import concourse.mybir as mybir
import concourse.tile as tile
from concourse.bass_utils import run_bass_kernel_spmd
from concourse.masks import make_identity

F32 = mybir.dt.float32
F32R = mybir.dt.float32r
Exp = mybir.ActivationFunctionType.Exp

NCORES = 8
B, T, C, H = 4, 2048, 1024, 16
HD = C // H          # 64
HL = H // NCORES     # 2 heads per core
D2 = HL * HD         # 128 rows of local head-pair dims
TB = T               # tokens per batch
NKC = C // 128       # 8 contraction chunks
NCH = TB // 512      # 4 tq chunks per batch
NTK = TB // 128      # 16 tk tiles per batch
PIECE = TB // NCORES  # 256 tokens per (batch, core) piece after AllToAll

_CACHE = {}


def _build(sim=False):
    nc = bacc.Bacc("TRN2", target_bir_lowering=False, debug=False,
                   num_devices=1 if sim else NCORES)
    xt = nc.dram_tensor("xt", [C, B * T], F32R, kind="ExternalInput").ap()
    wq = nc.dram_tensor("wq", [C, D2], F32R, kind="ExternalInput").ap()
    wk = nc.dram_tensor("wk", [C, D2], F32R, kind="ExternalInput").ap()
    wv = nc.dram_tensor("wv", [C, D2], F32R, kind="ExternalInput").ap()
    wp = nc.dram_tensor("wp", [C, C], F32R, kind="ExternalInput").ap()
    bqkv = nc.dram_tensor("bqkv", [D2, 3], F32, kind="ExternalInput").ap()
    bp = nc.dram_tensor("bp", [128, NKC], F32, kind="ExternalInput").ap()
    outp = nc.dram_tensor("outp", [C, B * PIECE], F32, kind="ExternalOutput").ap()

    inb = [nc.dram_tensor(f"inb{b}", [NCORES, D2, PIECE], F32R) for b in range(B)]
    outb = [nc.dram_tensor(f"outb{b}", [NCORES, D2, PIECE], F32R) for b in range(B)]

    with tile.TileContext(nc) as tc:
        with (
            tc.tile_pool(name="const", bufs=1) as cpool,
            tc.tile_pool(name="w", bufs=1) as wpool,
            tc.tile_pool(name="xt", bufs=16) as xpool,
            tc.tile_pool(name="qk", bufs=2) as qkpool,
            tc.tile_pool(name="vstg", bufs=2) as vstgpool,
            tc.tile_pool(name="vh", bufs=2) as vhpool,
            tc.tile_pool(name="pt", bufs=4) as ptpool,
            tc.tile_pool(name="small", bufs=3) as smallpool,
            tc.tile_pool(name="ofin", bufs=4) as ofinpool,
            tc.tile_pool(name="proj", bufs=3) as projpool,
            tc.tile_pool(name="otp", bufs=9) as otpool,
            tc.tile_pool(name="mm", bufs=2, space="PSUM") as mmps,
            tc.tile_pool(name="s", bufs=2, space="PSUM") as sps,
            tc.tile_pool(name="o", bufs=1, space="PSUM") as ops,
            
        ):
            # ---- constants ----
            ident32 = cpool.tile([128, 128], F32)
            make_identity(nc, ident32[:])
            idr = cpool.tile([128, 128], F32R)
            mask32 = cpool.tile([128, 512], F32)
            masks = cpool.tile([128, 4 * 512], F32R)
            ones32 = cpool.tile([128, 16], F32)
            ones64 = cpool.tile([1, 64], F32)
            onesr = cpool.tile([1, 64], F32R)
            nc.gpsimd.memset(ones32[:], 1.0)
            nc.gpsimd.memset(ones64[:], 1.0)
            with nc.allow_low_precision(reason="f32r operand staging"):
                nc.vector.tensor_copy(idr[:], ident32[:])
                nc.vector.tensor_copy(onesr[:], ones64[:])
                for m in range(4):
                    nc.gpsimd.memset(mask32[:], 0.0)
                    # keep where tq_local >= tk_local + 128*m
                    nc.gpsimd.affine_select(
                        out=mask32[:], in_=mask32[:],
                        compare_op=mybir.AluOpType.is_ge, fill=-1e30,
                        base=-128 * m, channel_multiplier=-1,
                        pattern=[[1, 512]],
                    )
                    nc.vector.tensor_copy(masks[:, 512 * m:512 * (m + 1)],
                                          mask32[:])

            # ---- weights ----
            wq_sb = wpool.tile([128, NKC, D2], F32R)
            wk_sb = wpool.tile([128, NKC, D2], F32R)
            wv_sb = wpool.tile([128, NKC, D2], F32R)
            for t, d in ((wq_sb, wq), (wk_sb, wk), (wv_sb, wv)):
                nc.sync.dma_start(
                    t[:], d.rearrange("(kc p) m -> p kc m", p=128))
            wp_sb = wpool.tile([128, NKC, C], F32R)
            nc.sync.dma_start(
                wp_sb[:], wp.rearrange("(kc p) m -> p kc m", p=128))
            bqkv_sb = cpool.tile([D2, 3], F32)
            nc.sync.dma_start(bqkv_sb[:], bqkv)
            bp_sb = cpool.tile([128, NKC], F32)
            nc.sync.dma_start(bp_sb[:], bp)

            for b in range(B):
                g0 = b * TB
                # ---- qkv projections ----
                qT = qkpool.tile([D2, TB], F32R, tag="qT")
                kT = qkpool.tile([D2, TB], F32R, tag="kT")
                vT = vstgpool.tile([D2, TB], F32)
                for n in range(NCH):
                    xts = []
                    for kc in range(NKC):
                        xtile = xpool.tile([128, 512], F32R)
                        nc.sync.dma_start(
                            xtile[:],
                            xt[128 * kc:128 * (kc + 1),
                               g0 + 512 * n:g0 + 512 * (n + 1)])
                        xts.append(xtile)
                    for w_sb, col in ((wq_sb, 0), (wk_sb, 1), (wv_sb, 2)):
                        ps = mmps.tile([128, 512], F32, tag="ps")
                        for kc in range(NKC):
                            nc.tensor.matmul(
                                ps[:], w_sb[:, kc, :],
                                xts[kc][:], start=(kc == 0),
                                stop=(kc == NKC - 1))
                        dst = (qT, kT, vT)[col]
                        with nc.allow_low_precision(reason="f32r qkv"):
                            nc.vector.tensor_scalar_add(
                                dst[:, 512 * n:512 * (n + 1)], ps[:],
                                bqkv_sb[:, col:col + 1])

                # ---- v transposes: vT [d2, T] -> per-head v [T, 65] tiles ----
                vh = [vhpool.tile([128, NTK * 65], F32R, tag=f"vh{h}",
                                  name=f"vh{h}") for h in range(HL)]
                for h in range(HL):
                    with nc.allow_low_precision(reason="f32r v ones"):
                        nc.vector.tensor_copy(vh[h][:, 64::65], ones32[:])
                    for tk in range(NTK):
                        vt_ps = mmps.tile([128, 64], F32, tag="ps", name="vt_ps")
                        nc.tensor.transpose(
                            vt_ps[:],
                            vT[64 * h:64 * (h + 1), 128 * tk:128 * (tk + 1)],
                            ident32[64 * h:64 * (h + 1), 64 * h:64 * (h + 1)])
                        with nc.allow_low_precision(reason="f32r v"):
                            nc.vector.tensor_copy(
                                vh[h][:, 65 * tk:65 * tk + 64], vt_ps[:])

                # ---- attention per tq-chunk ----
                for j in range(NCH):
                    o_ps = [ops.tile([65, 512], F32, tag=f"o{h}", name=f"o{h}")
                            for h in range(HL)]
                    ktop = 4 * j + 4
                    for tk in range(ktop):
                        m = tk - 4 * j
                        s_ps = sps.tile([128, 1024], F32, tag="s_ps")
                        if m >= 0:
                            for h in range(HL):
                                nc.tensor.matmul(
                                    s_ps[:, 512 * h:512 * (h + 1)], idr[:],
                                    masks[:, 512 * m:512 * (m + 1)],
                                    start=True, stop=False)
                        # K=64 pair at row groups (0,0)/(64,0) -> concurrent
                        for h in range(HL):
                            nc.tensor.matmul(
                                s_ps[:, 512 * h:512 * (h + 1)],
                                kT[64 * h:64 * (h + 1),
                                   128 * tk:128 * (tk + 1)],
                                qT[64 * h:64 * (h + 1),
                                   512 * j:512 * (j + 1)],
                                start=(m < 0), stop=True)
                        pt = ptpool.tile([128, 1024], F32R, tag="pt")
                        nc.scalar.activation(pt[:], s_ps[:], Exp)
                        for h in range(HL):
                            nc.tensor.matmul(
                                o_ps[h][:], vh[h][:, 65 * tk:65 * (tk + 1)],
                                pt[:, 512 * h:512 * (h + 1)],
                                start=(tk == 0), stop=(tk == ktop - 1))
                    for h in range(HL):
                        o_sb = smallpool.tile([65, 512], F32, tag="osb2")
                        nc.vector.tensor_copy(o_sb[:], o_ps[h][:])
                        r_sb = smallpool.tile([1, 512], F32R, tag="r")
                        with nc.allow_low_precision(reason="softmax denom"):
                            nc.vector.reciprocal(r_sb[:], o_sb[64:65, :])
                        rb_ps = mmps.tile([64, 512], F32, tag="ps", name="rb_ps")
                        nc.tensor.matmul(rb_ps[:], onesr[:], r_sb[:],
                                         start=True, stop=True)
                        rb_sb = smallpool.tile([64, 512], F32, tag="rb")
                        nc.vector.tensor_copy(rb_sb[:], rb_ps[:])
                        ofin = ofinpool.tile([64, 512], F32R)
                        with nc.allow_low_precision(reason="f32r O"):
                            nc.gpsimd.tensor_mul(ofin[:], o_sb[0:64, :],
                                                 rb_sb[:])
                        for half in range(2):
                            s8 = 2 * j + half
                            nc.sync.dma_start(
                                inb[b].ap()[s8, 64 * h:64 * (h + 1), :],
                                ofin[:, 256 * half:256 * (half + 1)])

                # ---- AllToAll: head-shards -> token-shards ----
                if sim:
                    # stand-in with comparable cost for the cost-model sim
                    nc.sync.dma_start(outb[b].ap(), inb[b].ap())
                else:
                    nc.gpsimd.collective_compute(
                        "AllToAll", mybir.AluOpType.bypass,
                        replica_groups=[list(range(NCORES))],
                        ins=[inb[b].ap().opt()], outs=[outb[b].ap().opt()],
                    )

                # ---- out projection (column-parallel, out^T) ----
                ots = []
                for s8 in range(NCORES):
                    ot = otpool.tile([128, PIECE], F32R, tag="ot")
                    nc.sync.dma_start(ot[:], outb[b].ap()[s8])
                    ots.append(ot)
                for mcol in range(NKC):
                    pp = mmps.tile([128, PIECE], F32, tag="ps")
                    for s8 in range(NCORES):
                        nc.tensor.matmul(
                            pp[:],
                            wp_sb[:, s8, 128 * mcol:128 * (mcol + 1)],
                            ots[s8][:], start=(s8 == 0),
                            stop=(s8 == NCORES - 1))
                    osb = projpool.tile([128, PIECE], F32, tag="osb")
                    nc.vector.tensor_scalar_add(osb[:], pp[:],
                                                bp_sb[:, mcol:mcol + 1])
                    nc.sync.dma_start(
                        outp[128 * mcol:128 * (mcol + 1),
                             PIECE * b:PIECE * (b + 1)], osb[:])
    nc.compile()
    return nc


def _get_nc():
    if "nc" not in _CACHE:
        _CACHE["nc"] = _build()
    return _CACHE["nc"]


def kernel(x, W_qkv, b_qkv, W_proj, b_proj):
    x = np.asarray(x, dtype=np.float32)
    W_qkv = np.asarray(W_qkv, dtype=np.float32)
    b_qkv = np.asarray(b_qkv, dtype=np.float32)
    W_proj = np.asarray(W_proj, dtype=np.float32)
    b_proj = np.asarray(b_proj, dtype=np.float32)

    scale = 1.0 / np.sqrt(HD)
    xt = np.ascontiguousarray(x.reshape(B * T, C).T)          # [C, B*T]
    wp = np.ascontiguousarray(W_proj)                          # [C, C]
    bp = np.ascontiguousarray(b_proj.reshape(NKC, 128).T)      # [128, 8]

    qw = W_qkv[:, 0:C]
    kw = W_qkv[:, C:2 * C]
    vw = W_qkv[:, 2 * C:3 * C]
    qb, kb, vb = b_qkv[0:C], b_qkv[C:2 * C], b_qkv[2 * C:3 * C]

    in_maps = []
    for c in range(NCORES):
        cols = slice(2 * c * HD, (2 * c + 2) * HD)  # this core's 128 dims
        bq = np.stack([qb[cols] * scale, kb[cols], vb[cols]], axis=1)  # [128,3]
        in_maps.append({
            "xt": xt,
            "wq": np.ascontiguousarray(qw[:, cols] * scale),
            "wk": np.ascontiguousarray(kw[:, cols]),
            "wv": np.ascontiguousarray(vw[:, cols]),
            "wp": wp,
            "bqkv": np.ascontiguousarray(bq),
            "bp": bp,
        })

    nc = _get_nc()
    _CACHE["last_in_maps"] = in_maps
    res = run_bass_kernel_spmd(nc, in_maps, core_ids=list(range(NCORES)))

    # outp[c]: [C, B*PIECE] (cols: b-major, then 256 tokens of piece c)
    allo = np.stack([res.results[c]["outp"] for c in range(NCORES)])
    allo = allo.reshape(NCORES, C, B, PIECE)       # [c, ch, b, u]
    out = allo.transpose(2, 0, 3, 1).reshape(B, T, C)
    return np.ascontiguousarray(out)


# revision 21
# speedup vs baseline: 17801.7635x; 17801.7635x over previous
"""Causal self-attention (B=4, T=2048, C=1024, H=16) on 8 trn2 NeuronCores.

Sharding: head-pair parallel. Core c owns heads {2c, 2c+1} for all 4 batches.
 - host: x is pre-transposed to xT [C, B*T]; W_qkv is pre-sliced per core into
   wq/wk/wv [C, 128] (2 heads x 64, softmax scale folded into wq), W_proj and
   biases broadcast.
 - device per core: qkv projections as fp32r matmuls producing qT/kT [d2, T]
   (d on partitions) and vT [d2, T]; vT is PE-transposed per 128-tile into
   v [T, 64]-per-head tiles with an appended ones column.
 - attention in S^T orientation: S^T[tk, tq] = kT.T@qT tiles [128, 512] with the
   causal mask preloaded into PSUM via an identity matmul; softmax without
   max-subtraction (|S| <= ~20, safe in fp32): P^T = exp(S^T) on ScalarE
   (PSUM->SBUF, rounded to f32r). O-matmul lhsT = [v_h | ones] (M=65) yields
   both O^T[d, tq] and the denominator row l in one pass. Normalize via
   reciprocal + K=1 broadcast matmul + DVE multiply.
 - per-batch AllToAll (1 MB/rank) reshards O^T from head-shards to
   token-shards; column-parallel out-projection with fused bias produces
   out^T [C, 1024 tokens] per core; host reassembles.
"""
import numpy as np
import concourse.bacc as bacc
import concourse.mybir as mybir
import concourse.tile as tile
from concourse.bass_utils import run_bass_kernel_spmd
from concourse.masks import make_identity

F32 = mybir.dt.float32
F32R = mybir.dt.float32r
Exp = mybir.ActivationFunctionType.Exp

NCORES = 8
B, T, C, H = 4, 2048, 1024, 16
HD = C // H          # 64
HL = H // NCORES     # 2 heads per core
D2 = HL * HD         # 128 rows of local head-pair dims
TB = T               # tokens per batch
NKC = C // 128       # 8 contraction chunks
NCH = TB // 512      # 4 tq chunks per batch
NTK = TB // 128      # 16 tk tiles per batch
PIECE = TB // NCORES  # 256 tokens per (batch, core) piece after AllToAll

_CACHE = {}


def _build(sim=False):
    nc = bacc.Bacc("TRN2", target_bir_lowering=False, debug=False,
                   num_devices=1 if sim else NCORES)
    xt = nc.dram_tensor("xt", [C, B * T], F32R, kind="ExternalInput").ap()
    wq = nc.dram_tensor("wq", [C, D2], F32R, kind="ExternalInput").ap()
    wk = nc.dram_tensor("wk", [C, D2], F32R, kind="ExternalInput").ap()
    wv = nc.dram_tensor("wv", [C, D2], F32R, kind="ExternalInput").ap()
    wp = nc.dram_tensor("wp", [C, C], F32R, kind="ExternalInput").ap()
    bqkv = nc.dram_tensor("bqkv", [D2, 3], F32, kind="ExternalInput").ap()
    bp = nc.dram_tensor("bp", [128, NKC], F32, kind="ExternalInput").ap()
    outp = nc.dram_tensor("outp", [C, B * PIECE], F32, kind="ExternalOutput").ap()

    inb = [nc.dram_tensor(f"inb{b}", [NCORES, D2, PIECE], F32R) for b in range(B)]
    outb = [nc.dram_tensor(f"outb{b}", [NCORES, D2, PIECE], F32R) for b in range(B)]

    with tile.TileContext(nc) as tc:
        with (
            tc.tile_pool(name="const", bufs=1) as cpool,
            tc.tile_pool(name="w", bufs=1) as wpool,
            tc.tile_pool(name="xt", bufs=16) as xpool,
            tc.tile_pool(name="qk", bufs=2) as qkpool,
            tc.tile_pool(name="vstg", bufs=2) as vstgpool,
            tc.tile_pool(name="vh", bufs=2) as vhpool,
            tc.tile_pool(name="pt", bufs=4) as ptpool,
            tc.tile_pool(name="small", bufs=3) as smallpool,
            tc.tile_pool(name="ofin", bufs=4) as ofinpool,
            tc.tile_pool(name="proj", bufs=3) as projpool,
            tc.tile_pool(name="otp", bufs=9) as otpool,
            tc.tile_pool(name="mm", bufs=2, space="PSUM") as mmps,
            tc.tile_pool(name="s", bufs=2, space="PSUM") as sps,
            tc.tile_pool(name="o", bufs=1, space="PSUM") as ops,
            
        ):
            # ---- constants ----
            ident32 = cpool.tile([128, 128], F32)
            make_identity(nc, ident32[:])
            idr = cpool.tile([128, 128], F32R)
            mask32 = cpool.tile([128, 512], F32)
            masks = cpool.tile([128, 4 * 512], F32R)
            ones32 = cpool.tile([128, 16], F32)
            ones64 = cpool.tile([1, 64], F32)
            onesr = cpool.tile([1, 64], F32R)
            nc.gpsimd.memset(ones32[:], 1.0)
            nc.gpsimd.memset(ones64[:], 1.0)
            with nc.allow_low_precision(reason="f32r operand staging"):
                nc.vector.tensor_copy(idr[:], ident32[:])
                nc.vector.tensor_copy(onesr[:], ones64[:])
                for m in range(4):
                    nc.gpsimd.memset(mask32[:], 0.0)
                    # keep where tq_local >= tk_local + 128*m
                    nc.gpsimd.affine_select(
                        out=mask32[:], in_=mask32[:],
                        compare_op=mybir.AluOpType.is_ge, fill=-1e30,
                        base=-128 * m, channel_multiplier=-1,
                        pattern=[[1, 512]],
                    )
                    nc.vector.tensor_copy(masks[:, 512 * m:512 * (m + 1)],
                                          mask32[:])

            # ---- weights ----
            wq_sb = wpool.tile([128, NKC, D2], F32R)
            wk_sb = wpool.tile([128, NKC, D2], F32R)
            wv_sb = wpool.tile([128, NKC, D2], F32R)
            for t, d in ((wq_sb, wq), (wk_sb, wk), (wv_sb, wv)):
                nc.sync.dma_start(
                    t[:], d.rearrange("(kc p) m -> p kc m", p=128))
            wp_sb = wpool.tile([128, NKC, C], F32R)
            nc.sync.dma_start(
                wp_sb[:], wp.rearrange("(kc p) m -> p kc m", p=128))
            bqkv_sb = cpool.tile([D2, 3], F32)
            nc.sync.dma_start(bqkv_sb[:], bqkv)
            bp_sb = cpool.tile([128, NKC], F32)
            nc.sync.dma_start(bp_sb[:], bp)

            for b in range(B):
                g0 = b * TB
                # ---- qkv projections ----
                qT = qkpool.tile([D2, TB], F32R, tag="qT")
                kT = qkpool.tile([D2, TB], F32R, tag="kT")
                vT = vstgpool.tile([D2, TB], F32)
                for n in range(NCH):
                    xts = []
                    for kc in range(NKC):
                        xtile = xpool.tile([128, 512], F32R)
                        nc.sync.dma_start(
                            xtile[:],
                            xt[128 * kc:128 * (kc + 1),
                               g0 + 512 * n:g0 + 512 * (n + 1)])
                        xts.append(xtile)
                    for w_sb, col in ((wq_sb, 0), (wk_sb, 1), (wv_sb, 2)):
                        ps = mmps.tile([128, 512], F32, tag="ps")
                        for kc in range(NKC):
                            nc.tensor.matmul(
                                ps[:], w_sb[:, kc, :],
                                xts[kc][:], start=(kc == 0),
                                stop=(kc == NKC - 1))
                        dst = (qT, kT, vT)[col]
                        with nc.allow_low_precision(reason="f32r qkv"):
                            nc.vector.tensor_scalar_add(
                                dst[:, 512 * n:512 * (n + 1)], ps[:],
                                bqkv_sb[:, col:col + 1])

                # ---- v transposes: vT [d2, T] -> per-head v [T, 65] tiles ----
                vh = [vhpool.tile([128, NTK * 65], F32R, tag=f"vh{h}",
                                  name=f"vh{h}") for h in range(HL)]
                for h in range(HL):
                    with nc.allow_low_precision(reason="f32r v ones"):
                        nc.vector.tensor_copy(vh[h][:, 64::65], ones32[:])
                    for tk in range(NTK):
                        vt_ps = mmps.tile([128, 64], F32, tag="ps", name="vt_ps")
                        nc.tensor.transpose(
                            vt_ps[:],
                            vT[64 * h:64 * (h + 1), 128 * tk:128 * (tk + 1)],
                            ident32[64 * h:64 * (h + 1), 64 * h:64 * (h + 1)])
                        with nc.allow_low_precision(reason="f32r v"):
                            nc.vector.tensor_copy(
                                vh[h][:, 65 * tk:65 * tk + 64], vt_ps[:])

                # ---- attention per tq-chunk ----
                for j in range(NCH):
                    o_ps = [ops.tile([65, 512], F32, tag=f"o{h}", name=f"o{h}")
                            for h in range(HL)]
                    ktop = 4 * j + 4
                    for tk in range(ktop):
                        m = tk - 4 * j
                        # cols [0, z) of this tile are fully causal-masked
                        z = 128 * m if m > 0 else 0
                        w = 512 - z
                        s_ps = sps.tile([128, 1024], F32, tag="s_ps")
                        if m >= 0:
                            for h in range(HL):
                                nc.tensor.matmul(
                                    s_ps[:, 512 * h + z:512 * (h + 1)],
                                    idr[:],
                                    masks[:, 512 * m + z:512 * (m + 1)],
                                    start=True, stop=False)
                        # K=64 pair at row groups (0,0)/(64,0) -> concurrent
                        for h in range(HL):
                            nc.tensor.matmul(
                                s_ps[:, 512 * h + z:512 * (h + 1)],
                                kT[64 * h:64 * (h + 1),
                                   128 * tk:128 * (tk + 1)],
                                qT[64 * h:64 * (h + 1),
                                   512 * j + z:512 * (j + 1)],
                                start=(m < 0), stop=True)
                        pt = ptpool.tile([128, 1024], F32R, tag="pt")
                        if z:
                            exp_src = s_ps[:].rearrange(
                                "p (g c) -> p g c", g=2)[:, :, z:]
                            exp_dst = pt[:].rearrange(
                                "p (g c) -> p g c", g=2)[:, :, z:]
                            nc.scalar.activation(exp_dst, exp_src, Exp)
                        else:
                            nc.scalar.activation(pt[:], s_ps[:], Exp)
                        for h in range(HL):
                            nc.tensor.matmul(
                                o_ps[h][0:65, z:512],
                                vh[h][:, 65 * tk:65 * (tk + 1)],
                                pt[:, 512 * h + z:512 * (h + 1)],
                                start=(tk == 0), stop=(tk == ktop - 1))
                    for h in range(HL):
                        o_sb = smallpool.tile([65, 512], F32, tag="osb2")
                        nc.vector.tensor_copy(o_sb[:], o_ps[h][:])
                        r_sb = smallpool.tile([1, 512], F32R, tag="r")
                        with nc.allow_low_precision(reason="softmax denom"):
                            nc.vector.reciprocal(r_sb[:], o_sb[64:65, :])
                        rb_ps = mmps.tile([64, 512], F32, tag="ps", name="rb_ps")
                        nc.tensor.matmul(rb_ps[:], onesr[:], r_sb[:],
                                         start=True, stop=True)
                        rb_sb = smallpool.tile([64, 512], F32, tag="rb")
                        nc.vector.tensor_copy(rb_sb[:], rb_ps[:])
                        ofin = ofinpool.tile([64, 512], F32R)
                        with nc.allow_low_precision(reason="f32r O"):
                            nc.gpsimd.tensor_mul(ofin[:], o_sb[0:64, :],
                                                 rb_sb[:])
                        for half in range(2):
                            s8 = 2 * j + half
                            nc.sync.dma_start(
                                inb[b].ap()[s8, 64 * h:64 * (h + 1), :],
                                ofin[:, 256 * half:256 * (half + 1)])

                # ---- AllToAll: head-shards -> token-shards ----
                if sim:
                    # stand-in with comparable cost for the cost-model sim
                    nc.sync.dma_start(outb[b].ap(), inb[b].ap())
                else:
                    nc.gpsimd.collective_compute(
                        "AllToAll", mybir.AluOpType.bypass,
                        replica_groups=[list(range(NCORES))],
                        ins=[inb[b].ap().opt()], outs=[outb[b].ap().opt()],
                    )

                # ---- out projection (column-parallel, out^T) ----
                ots = []
                for s8 in range(NCORES):
                    ot = otpool.tile([128, PIECE], F32R, tag="ot")
                    nc.sync.dma_start(ot[:], outb[b].ap()[s8])
                    ots.append(ot)
                for mcol in range(NKC):
                    pp = mmps.tile([128, PIECE], F32, tag="ps")
                    for s8 in range(NCORES):
                        nc.tensor.matmul(
                            pp[:],
                            wp_sb[:, s8, 128 * mcol:128 * (mcol + 1)],
                            ots[s8][:], start=(s8 == 0),
                            stop=(s8 == NCORES - 1))
                    osb = projpool.tile([128, PIECE], F32, tag="osb")
                    nc.vector.tensor_scalar_add(osb[:], pp[:],
                                                bp_sb[:, mcol:mcol + 1])
                    nc.sync.dma_start(
                        outp[128 * mcol:128 * (mcol + 1),
                             PIECE * b:PIECE * (b + 1)], osb[:])
    nc.compile()
    return nc


def _get_nc():
    if "nc" not in _CACHE:
        _CACHE["nc"] = _build()
    return _CACHE["nc"]


def kernel(x, W_qkv, b_qkv, W_proj, b_proj):
    x = np.asarray(x, dtype=np.float32)
    W_qkv = np.asarray(W_qkv, dtype=np.float32)
    b_qkv = np.asarray(b_qkv, dtype=np.float32)
    W_proj = np.asarray(W_proj, dtype=np.float32)
    b_proj = np.asarray(b_proj, dtype=np.float32)

    scale = 1.0 / np.sqrt(HD)
    xt = np.ascontiguousarray(x.reshape(B * T, C).T)          # [C, B*T]
    wp = np.ascontiguousarray(W_proj)                          # [C, C]
    bp = np.ascontiguousarray(b_proj.reshape(NKC, 128).T)      # [128, 8]

    qw = W_qkv[:, 0:C]
    kw = W_qkv[:, C:2 * C]
    vw = W_qkv[:, 2 * C:3 * C]
    qb, kb, vb = b_qkv[0:C], b_qkv[C:2 * C], b_qkv[2 * C:3 * C]

    in_maps = []
    for c in range(NCORES):
        cols = slice(2 * c * HD, (2 * c + 2) * HD)  # this core's 128 dims
        bq = np.stack([qb[cols] * scale, kb[cols], vb[cols]], axis=1)  # [128,3]
        in_maps.append({
            "xt": xt,
            "wq": np.ascontiguousarray(qw[:, cols] * scale),
            "wk": np.ascontiguousarray(kw[:, cols]),
            "wv": np.ascontiguousarray(vw[:, cols]),
            "wp": wp,
            "bqkv": np.ascontiguousarray(bq),
            "bp": bp,
        })

    nc = _get_nc()
    _CACHE["last_in_maps"] = in_maps
    res = run_bass_kernel_spmd(nc, in_maps, core_ids=list(range(NCORES)))

    # outp[c]: [C, B*PIECE] (cols: b-major, then 256 tokens of piece c)
    allo = np.stack([res.results[c]["outp"] for c in range(NCORES)])
    allo = allo.reshape(NCORES, C, B, PIECE)       # [c, ch, b, u]
    out = allo.transpose(2, 0, 3, 1).reshape(B, T, C)
    return np.ascontiguousarray(out)


# revision 25
# speedup vs baseline: 18029.7472x; 1.0128x over previous
"""Causal self-attention (B=4, T=2048, C=1024, H=16) on 8 trn2 NeuronCores.

Sharding: head-pair parallel. Core c owns heads {2c, 2c+1} for all 4 batches.
 - host: x is pre-transposed to xT [C, B*T]; W_qkv is pre-sliced per core into
   wq/wk/wv [C, 128] (2 heads x 64, softmax scale folded into wq), W_proj and
   biases broadcast.
 - device per core: qkv projections as fp32r matmuls producing qT/kT [d2, T]
   (d on partitions) and vT [d2, T]; vT is PE-transposed per 128-tile into
   v [T, 64]-per-head tiles with an appended ones column.
 - attention in S^T orientation: S^T[tk, tq] = kT.T@qT tiles [128, 512] with the
   causal mask preloaded into PSUM via an identity matmul; softmax without
   max-subtraction (|S| <= ~20, safe in fp32): P^T = exp(S^T) on ScalarE
   (PSUM->SBUF, rounded to f32r). O-matmul lhsT = [v_h | ones] (M=65) yields
   both O^T[d, tq] and the denominator row l in one pass. Normalize via
   reciprocal + K=1 broadcast matmul + DVE multiply.
 - per-batch AllToAll (1 MB/rank) reshards O^T from head-shards to
   token-shards; column-parallel out-projection with fused bias produces
   out^T [C, 1024 tokens] per core; host reassembles.
"""
import numpy as np
import concourse.bacc as bacc
import concourse.mybir as mybir
import concourse.tile as tile
from concourse.bass_utils import run_bass_kernel_spmd
from concourse.masks import make_identity

F32 = mybir.dt.float32
F32R = mybir.dt.float32r
Exp = mybir.ActivationFunctionType.Exp

NCORES = 8
B, T, C, H = 4, 2048, 1024, 16
HD = C // H          # 64
HL = H // NCORES     # 2 heads per core
D2 = HL * HD         # 128 rows of local head-pair dims
TB = T               # tokens per batch
NKC = C // 128       # 8 contraction chunks
NCH = TB // 512      # 4 tq chunks per batch
NTK = TB // 128      # 16 tk tiles per batch
PIECE = TB // NCORES  # 256 tokens per (batch, core) piece after AllToAll

_CACHE = {}


def _build(sim=False):
    nc = bacc.Bacc("TRN2", target_bir_lowering=False, debug=False,
                   num_devices=1 if sim else NCORES)
    xt = nc.dram_tensor("xt", [C, B * T], F32R, kind="ExternalInput").ap()
    wq = nc.dram_tensor("wq", [C, D2], F32R, kind="ExternalInput").ap()
    wk = nc.dram_tensor("wk", [C, D2], F32R, kind="ExternalInput").ap()
    wv = nc.dram_tensor("wv", [C, D2], F32R, kind="ExternalInput").ap()
    wp = nc.dram_tensor("wp", [C, C], F32R, kind="ExternalInput").ap()
    bqkv = nc.dram_tensor("bqkv", [D2, 3], F32, kind="ExternalInput").ap()
    bp = nc.dram_tensor("bp", [128, NKC], F32, kind="ExternalInput").ap()
    outp = nc.dram_tensor("outp", [C, B * PIECE], F32, kind="ExternalOutput").ap()

    inb = [nc.dram_tensor(f"inb{b}", [NCORES, D2, PIECE], F32R) for b in range(B)]
    outb = [nc.dram_tensor(f"outb{b}", [NCORES, D2, PIECE], F32R) for b in range(B)]

    with tile.TileContext(nc) as tc:
        with (
            tc.tile_pool(name="const", bufs=1) as cpool,
            tc.tile_pool(name="w", bufs=1) as wpool,
            tc.tile_pool(name="xt", bufs=16) as xpool,
            tc.tile_pool(name="qk", bufs=2) as qkpool,
            tc.tile_pool(name="vstg", bufs=1) as vstgpool,
            tc.tile_pool(name="vh", bufs=2) as vhpool,
            tc.tile_pool(name="pt", bufs=5) as ptpool,
            tc.tile_pool(name="small", bufs=3) as smallpool,
            tc.tile_pool(name="ofin", bufs=4) as ofinpool,
            tc.tile_pool(name="proj", bufs=3) as projpool,
            tc.tile_pool(name="otp", bufs=9) as otpool,
            tc.tile_pool(name="mm", bufs=2, space="PSUM") as mmps,
            tc.tile_pool(name="s", bufs=2, space="PSUM") as sps,
            tc.tile_pool(name="o", bufs=1, space="PSUM") as ops,
            
        ):
            # ---- constants ----
            ident32 = cpool.tile([128, 128], F32)
            make_identity(nc, ident32[:])
            idr = cpool.tile([128, 128], F32R)
            mask32 = cpool.tile([128, 512], F32)
            masks = cpool.tile([128, 4 * 512], F32R)
            ones32 = cpool.tile([128, 16], F32)
            ones64 = cpool.tile([1, 64], F32)
            onesr = cpool.tile([1, 64], F32R)
            nc.gpsimd.memset(ones32[:], 1.0)
            nc.gpsimd.memset(ones64[:], 1.0)
            with nc.allow_low_precision(reason="f32r operand staging"):
                nc.vector.tensor_copy(idr[:], ident32[:])
                nc.vector.tensor_copy(onesr[:], ones64[:])
                for m in range(4):
                    nc.gpsimd.memset(mask32[:], 0.0)
                    # keep where tq_local >= tk_local + 128*m
                    nc.gpsimd.affine_select(
                        out=mask32[:], in_=mask32[:],
                        compare_op=mybir.AluOpType.is_ge, fill=-1e30,
                        base=-128 * m, channel_multiplier=-1,
                        pattern=[[1, 512]],
                    )
                    nc.vector.tensor_copy(masks[:, 512 * m:512 * (m + 1)],
                                          mask32[:])

            # ---- weights ----
            wq_sb = wpool.tile([128, NKC, D2], F32R)
            wk_sb = wpool.tile([128, NKC, D2], F32R)
            wv_sb = wpool.tile([128, NKC, D2], F32R)
            for t, d in ((wq_sb, wq), (wk_sb, wk), (wv_sb, wv)):
                nc.sync.dma_start(
                    t[:], d.rearrange("(kc p) m -> p kc m", p=128))
            wp_sb = wpool.tile([128, NKC, C], F32R)
            nc.sync.dma_start(
                wp_sb[:], wp.rearrange("(kc p) m -> p kc m", p=128))
            bqkv_sb = cpool.tile([D2, 3], F32)
            nc.sync.dma_start(bqkv_sb[:], bqkv)
            bp_sb = cpool.tile([128, NKC], F32)
            nc.sync.dma_start(bp_sb[:], bp)

            for b in range(B):
                g0 = b * TB
                # ---- qkv projections ----
                qT = qkpool.tile([D2, TB], F32R, tag="qT")
                kT = qkpool.tile([D2, TB], F32R, tag="kT")
                vT = vstgpool.tile([D2, TB], F32)
                for n in range(NCH):
                    xts = []
                    for kc in range(NKC):
                        xtile = xpool.tile([128, 512], F32R)
                        nc.sync.dma_start(
                            xtile[:],
                            xt[128 * kc:128 * (kc + 1),
                               g0 + 512 * n:g0 + 512 * (n + 1)])
                        xts.append(xtile)
                    for w_sb, col in ((wq_sb, 0), (wk_sb, 1), (wv_sb, 2)):
                        ps = mmps.tile([128, 512], F32, tag="ps")
                        for kc in range(NKC):
                            nc.tensor.matmul(
                                ps[:], w_sb[:, kc, :],
                                xts[kc][:], start=(kc == 0),
                                stop=(kc == NKC - 1))
                        dst = (qT, kT, vT)[col]
                        with nc.allow_low_precision(reason="f32r qkv"):
                            nc.vector.tensor_scalar_add(
                                dst[:, 512 * n:512 * (n + 1)], ps[:],
                                bqkv_sb[:, col:col + 1])

                # ---- v transposes: vT [d2, T] -> per-head v [T, 65] tiles ----
                vh = [vhpool.tile([128, NTK * 65], F32R, tag=f"vh{h}",
                                  name=f"vh{h}") for h in range(HL)]
                for h in range(HL):
                    with nc.allow_low_precision(reason="f32r v ones"):
                        nc.vector.tensor_copy(vh[h][:, 64::65], ones32[:])
                    for tk in range(NTK):
                        vt_ps = mmps.tile([128, 64], F32, tag="ps", name="vt_ps")
                        nc.tensor.transpose(
                            vt_ps[:],
                            vT[64 * h:64 * (h + 1), 128 * tk:128 * (tk + 1)],
                            ident32[64 * h:64 * (h + 1), 64 * h:64 * (h + 1)])
                        with nc.allow_low_precision(reason="f32r v"):
                            nc.vector.tensor_copy(
                                vh[h][:, 65 * tk:65 * tk + 64], vt_ps[:])

                # ---- attention per tq-chunk ----
                for j in range(NCH):
                    o_ps = [ops.tile([65, 512], F32, tag=f"o{h}", name=f"o{h}")
                            for h in range(HL)]
                    ktop = 4 * j + 4
                    for tk in range(ktop):
                        m = tk - 4 * j
                        # cols [0, z) of this tile are fully causal-masked
                        z = 128 * m if m > 0 else 0
                        w = 512 - z
                        s_ps = sps.tile([128, 1024], F32, tag="s_ps")
                        if m >= 0:
                            for h in range(HL):
                                nc.tensor.matmul(
                                    s_ps[:, 512 * h + z:512 * (h + 1)],
                                    idr[:],
                                    masks[:, 512 * m + z:512 * (m + 1)],
                                    start=True, stop=False)
                        # K=64 pair at row groups (0,0)/(64,0) -> concurrent
                        for h in range(HL):
                            nc.tensor.matmul(
                                s_ps[:, 512 * h + z:512 * (h + 1)],
                                kT[64 * h:64 * (h + 1),
                                   128 * tk:128 * (tk + 1)],
                                qT[64 * h:64 * (h + 1),
                                   512 * j + z:512 * (j + 1)],
                                start=(m < 0), stop=True)
                        pt = ptpool.tile([128, 1024], F32R, tag="pt")
                        if z:
                            exp_src = s_ps[:].rearrange(
                                "p (g c) -> p g c", g=2)[:, :, z:]
                            exp_dst = pt[:].rearrange(
                                "p (g c) -> p g c", g=2)[:, :, z:]
                            nc.scalar.activation(exp_dst, exp_src, Exp)
                        else:
                            nc.scalar.activation(pt[:], s_ps[:], Exp)
                        for h in range(HL):
                            nc.tensor.matmul(
                                o_ps[h][0:65, z:512],
                                vh[h][:, 65 * tk:65 * (tk + 1)],
                                pt[:, 512 * h + z:512 * (h + 1)],
                                start=(tk == 0), stop=(tk == ktop - 1))
                    for h in range(HL):
                        o_sb = smallpool.tile([65, 512], F32, tag="osb2")
                        nc.vector.tensor_copy(o_sb[:], o_ps[h][:])
                        r_sb = smallpool.tile([1, 512], F32R, tag="r")
                        with nc.allow_low_precision(reason="softmax denom"):
                            nc.vector.reciprocal(r_sb[:], o_sb[64:65, :])
                        rb_ps = mmps.tile([64, 512], F32, tag="ps", name="rb_ps")
                        nc.tensor.matmul(rb_ps[:], onesr[:], r_sb[:],
                                         start=True, stop=True)
                        rb_sb = smallpool.tile([64, 512], F32, tag="rb")
                        nc.vector.tensor_copy(rb_sb[:], rb_ps[:])
                        ofin = ofinpool.tile([64, 512], F32R)
                        with nc.allow_low_precision(reason="f32r O"):
                            nc.gpsimd.tensor_mul(ofin[:], o_sb[0:64, :],
                                                 rb_sb[:])
                        for half in range(2):
                            s8 = 2 * j + half
                            nc.sync.dma_start(
                                inb[b].ap()[s8, 64 * h:64 * (h + 1), :],
                                ofin[:, 256 * half:256 * (half + 1)])

                # ---- AllToAll: head-shards -> token-shards ----
                if sim:
                    # stand-in with comparable cost for the cost-model sim
                    nc.sync.dma_start(outb[b].ap(), inb[b].ap())
                else:
                    nc.gpsimd.collective_compute(
                        "AllToAll", mybir.AluOpType.bypass,
                        replica_groups=[list(range(NCORES))],
                        ins=[inb[b].ap().opt()], outs=[outb[b].ap().opt()],
                    )

                # ---- out projection (column-parallel, out^T) ----
                ots = []
                for s8 in range(NCORES):
                    ot = otpool.tile([128, PIECE], F32R, tag="ot")
                    nc.sync.dma_start(ot[:], outb[b].ap()[s8])
                    ots.append(ot)
                for mcol in range(NKC):
                    pp = mmps.tile([128, PIECE], F32, tag="ps")
                    for s8 in range(NCORES):
                        nc.tensor.matmul(
                            pp[:],
                            wp_sb[:, s8, 128 * mcol:128 * (mcol + 1)],
                            ots[s8][:], start=(s8 == 0),
                            stop=(s8 == NCORES - 1))
                    osb = projpool.tile([128, PIECE], F32, tag="osb")
                    nc.vector.tensor_scalar_add(osb[:], pp[:],
                                                bp_sb[:, mcol:mcol + 1])
                    nc.sync.dma_start(
                        outp[128 * mcol:128 * (mcol + 1),
                             PIECE * b:PIECE * (b + 1)], osb[:])
    nc.compile()
    return nc


def _get_nc():
    if "nc" not in _CACHE:
        _CACHE["nc"] = _build()
    return _CACHE["nc"]


def kernel(x, W_qkv, b_qkv, W_proj, b_proj):
    x = np.asarray(x, dtype=np.float32)
    W_qkv = np.asarray(W_qkv, dtype=np.float32)
    b_qkv = np.asarray(b_qkv, dtype=np.float32)
    W_proj = np.asarray(W_proj, dtype=np.float32)
    b_proj = np.asarray(b_proj, dtype=np.float32)

    scale = 1.0 / np.sqrt(HD)
    xt = np.ascontiguousarray(x.reshape(B * T, C).T)          # [C, B*T]
    wp = np.ascontiguousarray(W_proj)                          # [C, C]
    bp = np.ascontiguousarray(b_proj.reshape(NKC, 128).T)      # [128, 8]

    qw = W_qkv[:, 0:C]
    kw = W_qkv[:, C:2 * C]
    vw = W_qkv[:, 2 * C:3 * C]
    qb, kb, vb = b_qkv[0:C], b_qkv[C:2 * C], b_qkv[2 * C:3 * C]

    in_maps = []
    for c in range(NCORES):
        cols = slice(2 * c * HD, (2 * c + 2) * HD)  # this core's 128 dims
        bq = np.stack([qb[cols] * scale, kb[cols], vb[cols]], axis=1)  # [128,3]
        in_maps.append({
            "xt": xt,
            "wq": np.ascontiguousarray(qw[:, cols] * scale),
            "wk": np.ascontiguousarray(kw[:, cols]),
            "wv": np.ascontiguousarray(vw[:, cols]),
            "wp": wp,
            "bqkv": np.ascontiguousarray(bq),
            "bp": bp,
        })

    nc = _get_nc()
    _CACHE["last_in_maps"] = in_maps
    res = run_bass_kernel_spmd(nc, in_maps, core_ids=list(range(NCORES)))

    # outp[c]: [C, B*PIECE] (cols: b-major, then 256 tokens of piece c)
    allo = np.stack([res.results[c]["outp"] for c in range(NCORES)])
    allo = allo.reshape(NCORES, C, B, PIECE)       # [c, ch, b, u]
    out = allo.transpose(2, 0, 3, 1).reshape(B, T, C)
    return np.ascontiguousarray(out)
